# revision 17
# baseline (speedup 1.0000x reference)
"""Trainium2 Bass kernel for GNN message passing (APPR-style aggregation).

Computes: out = x + 0.15 * segment_sum(x[src], dst, num_segments=N)
for x [100000, 64] f32 and edge_index [2, 1600000] int64.

Strategy (8 NeuronCores, no collectives needed):
  - Host shards EDGES by destination-owner core (core c owns nodes
    [c*12500, (c+1)*12500)); within a core, edges are bucketed by
    128-node destination block and by source quadrant (x split into 4
    row-quadrants so dma_gather's int16 indices can address it). This
    makes the aggregation node-sharded from the start, so each core
    independently produces its slice of the output.
  - On device, per 128-node block: dma_gather of 0.15*x[src] rows (bf16,
    rows padded to 256 B) into SBUF, one gather per source quadrant,
    round-robined over all 4 SWDGE queues; build one-hot selection
    matrices S[e, j] = (dstcol_e == j) with a DVE is_equal against an
    iota tile, and accumulate S^T @ gathered into an f32 PSUM tile over
    all edge tiles of the block. Epilogue adds the f32 x slice and DMAs
    the block out.
  - All 8 cores run the same static graph: per-(block, quadrant) tile
    counts are maxed across cores; each core pads its edge list with
    valid index-0 entries up to the shared valid count and with -1
    entries (skipped by the gather) up to the tile boundary. Pad slots
    carry dstcol=255 which matches no one-hot column, so they contribute
    zero; slots the gather skips hold stale-but-finite SBUF data (the
    gather pool is memset once at startup).

  Perf notes (measured): the bottleneck is GPSIMD Q7 descriptor emission
  for the gathers (~3.5 ns/row with 4 SWDGE queues; 1 queue is ~3x
  slower). dma_gather is capped at 1024 indices per instruction.
  TensorE/VectorE/DMA all hide under the gather wall.
"""

import os
import sys
import types

import numpy as np

for _p in ("/opt/trn_rl_repo", "/root/.axon_site/_ro/trn_rl_repo"):
    if os.path.isdir(_p) and _p not in sys.path:
        sys.path.append(_p)

import ml_dtypes
import concourse.bass as bass
import concourse.mybir as mybir
import concourse.tile as tile
from concourse import bacc
from concourse.bass_utils import run_bass_kernel_spmd
from concourse.vector_clock import ScopedClock

WEIGHT = 0.15
N_NODES = 100000
D_FEAT = 64
N_CORES = 8
P = 128
NQUAD = 4
ROWPAD = 128  # gathered bf16 row padded to 128 elems = 256 B
NPC = N_NODES // N_CORES  # nodes per core
NBLK = (NPC + P - 1) // P  # 128-node dst blocks per core
NQROWS = N_NODES // NQUAD  # rows per source quadrant (must fit int16)

LAST_EXEC_TIME_NS = None

# Emit only up to the max-over-cores valid count per gather and pad the
# rest with -1 (skipped; slots keep stale-but-finite SBUF data). The
# CoreSim interpreter poisons unwritten regions of the gather's out view,
# so simulation-based tests set this to False.
TRAILING_SKIP = True

MAX_WAITS = 2  # this walrus build rejects instructions with more sync commands


def _patch_tile_drain():
    """This walrus build rejects >MAX_WAITS sync commands (waits+updates)
    on one instruction. Two patches: (a) the tail drain re-emits its waits
    as individual wait_ge instructions; (b) any scheduled instruction with
    too many waits gets the excess hoisted onto same-engine InstNoOps
    placed immediately before it."""
    if getattr(tile.TileContext, "_drain_patched", False):
        return

    def _drain_and_barrier(self, tick_clock, wait_clock):
        drain_inst = self.nc.sync.drain()
        wait_clock.add_sem_waits(
            drain_inst.ins, ScopedClock({None: tick_clock.global_clock})
        )
        si = drain_inst.ins.sync_info
        waits = list(si.on_wait) if si is not None else []
        if len(waits) > MAX_WAITS:
            drain_inst.ins.sync_info = mybir.SyncInfo(on_wait=[], on_update=[])
            handles = {h.name: h for h in wait_clock.sems.allocated().values()}
            for w in waits:
                self.nc.sync.wait_ge(handles[w.ant_name], w.wait_value)
            self.nc.sync.drain()
        self.nc.all_engine_barrier()
        popped = self.nc._tile_sem_poison_stack.pop()
        assert popped is self._sem_poison
        self.nc.clear_and_free_semaphores(list(self.sems.allocated().values()))
        self.nc.all_engine_barrier()

    orig_lower = tile.TileContext._lower_ordered_insts

    def _lower_ordered_insts(self, ordered):
        for bb_name, insts in ordered.items():
            new_list = []
            for inst in insts:
                si = getattr(inst, "sync_info", None)
                n_w = len(si.on_wait) if si is not None and si.on_wait else 0
                n_u = len(si.on_update) if si is not None and si.on_update else 0
                budget = max(0, MAX_WAITS - n_u)
                if (
                    n_w > budget
                    and type(inst).__name__.startswith("Inst")
                    and inst.engine is not None
                ):
                    waits = list(si.on_wait)
                    keep = waits[len(waits) - budget :] if budget else []
                    excess = waits[: len(waits) - budget]
                    for w in excess:
                        nop = mybir.InstNoOp(
                            name=self.nc.get_next_instruction_name(),
                            sync_info=mybir.SyncInfo(on_wait=[w], on_update=[]),
                            engine=inst.engine,
                            bass_nofuse=True,
                        )
                        new_list.append(nop)
                    inst.sync_info = mybir.SyncInfo(
                        on_wait=keep, on_update=list(si.on_update)
                    )
                new_list.append(inst)
            insts[:] = new_list
        return orig_lower(self, ordered)

    tile.TileContext._drain_and_barrier = _drain_and_barrier
    tile.TileContext._lower_ordered_insts = _lower_ordered_insts
    tile.TileContext._drain_patched = True


def _install_ntff_hook():
    """Register the NTFF profiling hook that this container's boot skips
    (antenv.axon_hooks missing). Only needed when tracing is requested."""
    if "antenv.axon_hooks" in sys.modules:
        return
    try:
        from trn_agent_boot.trn_boot import _ntff_profile_via_ctypes

        hook = _ntff_profile_via_ctypes("/opt/axon/libaxon_pjrt.so")
        if hook is None:
            return
        mod = types.ModuleType("antenv.axon_hooks")
        mod._hook = hook
        mod.get_axon_ntff_profile_hook = lambda: mod._hook
        mod.set_axon_ntff_profile_hook = lambda h: setattr(mod, "_hook", h)
        sys.modules["antenv.axon_hooks"] = mod
        import antenv

        antenv.axon_hooks = mod
    except Exception as e:  # profiling is optional
        print(f"ntff hook install failed: {e}", file=sys.stderr)


def _preprocess(x, edge_index):
    """Bucket edges per (core, dst-block, src-quadrant); build device
    input arrays in the slot order the device graph consumes."""
    src = np.asarray(edge_index[0]).astype(np.int64)
    dst = np.asarray(edge_index[1]).astype(np.int64)
    E = src.shape[0]

    core = dst // NPC
    local = dst - core * NPC
    blk = local >> 7
    col = local & 127
    quad = src // NQROWS
    loc = (src - quad * NQROWS).astype(np.int64)

    gkey = (core * NBLK + blk) * NQUAD + quad
    order = np.argsort(gkey, kind="stable")
    gkey_s = gkey[order]
    loc_s = loc[order]
    col_s = col[order]

    counts = np.bincount(gkey, minlength=N_CORES * NBLK * NQUAD).reshape(
        N_CORES, NBLK, NQUAD
    )
    maxc = counts.max(axis=0)  # [NBLK, NQUAD] max edges over cores
    tq = (maxc + P - 1) // P  # tiles per (block, quadrant)
    for b in range(NBLK):
        if tq[b].sum() == 0:
            tq[b, 0] = 1
    tiles = tq.sum(axis=1)
    t_total = int(tiles.sum())

    toff = np.zeros((NBLK, NQUAD), dtype=np.int64)
    toff.ravel()[1:] = np.cumsum(tq.ravel())[:-1]

    group_starts = np.zeros(N_CORES * NBLK * NQUAD + 1, dtype=np.int64)
    np.cumsum(counts.ravel(), out=group_starts[1:])
    j = np.arange(E) - group_starts[gkey_s]  # slot within group
    bq_s = gkey_s % (NBLK * NQUAD)
    core_s = gkey_s // (NBLK * NQUAD)
    base_tile = toff.ravel()[bq_s]

    # col array: [core, 128, t_total]; slot i -> lane i%128, tile base+i//128
    col_arr = np.full((N_CORES, P, t_total), 255.0, dtype=ml_dtypes.bfloat16)
    col_arr[core_s, j & 127, base_tile + (j >> 7)] = col_s.astype(
        ml_dtypes.bfloat16
    )

    # Gather chunks: dma_gather is capped at 8 tiles (1024 idxs); each
    # (b, q) group is split into <=8-tile chunks. Build the per-core,
    # per-chunk valid counts (fed to num_idxs_reg via a register) and the
    # idx arrays: each core's real edges, then -1s (skipped) to the tile
    # boundary. A chunk with zero valid entries for some core gets one
    # valid pad entry (index 0) so the gather ucode never sees count 0.
    t16_total = t_total * 8
    fill = -1 if TRAILING_SKIP else 0
    idx16 = np.full((N_CORES, 16, t16_total), fill, dtype=np.int16)
    idx16[core_s, j & 15, base_tile * 8 + (j >> 4)] = loc_s

    chunks = []  # (b, q, tile_offset_in_block, n_tiles, chunk_idx_global)
    vc_list = []  # per chunk: [N_CORES] valid counts
    for b in range(NBLK):
        for q in range(NQUAD):
            tn = int(tq[b, q])
            done = 0
            while done < tn:
                ch = min(8, tn - done)
                lo, hi = done * P, done * P + ch * P
                v = np.clip(counts[:, b, q] - lo, 0, ch * P)
                if TRAILING_SKIP:
                    need_pad = v == 0
                    if need_pad.any():
                        o16 = (int(toff[b, q]) + done) * 8
                        idx16[need_pad, 0, o16] = 0
                    v = np.maximum(v, 1)
                else:
                    v[:] = ch * P
                chunks.append((b, q, done, ch))
                vc_list.append(v)
                done += ch
    vc_arr = np.stack(vc_list, axis=1).astype(np.int32)  # [N_CORES, G]
    idx_arr = np.tile(idx16, (1, 8, 1))

    return idx_arr, col_arr, tq.astype(int), chunks, vc_arr, t_total


def _build_graph(tq, chunks, t_total):
    nc = bacc.Bacc(num_swdge_queues=4, dynamic_dma_scratch_size=65536)
    f32 = mybir.dt.float32
    bf16 = mybir.dt.bfloat16
    xq_p = [
        nc.declare_dram_parameter(f"xq{q}", [NQROWS, ROWPAD], bf16, isOutput=False)
        for q in range(NQUAD)
    ]
    xsl_p = nc.declare_dram_parameter("xsl", [NPC, D_FEAT], f32, isOutput=False)
    srci_p = nc.declare_dram_parameter(
        "srci", [P, t_total * 8], mybir.dt.int16, isOutput=False
    )
    n_chunks = len(chunks)
    vc_p = nc.declare_dram_parameter("vc", [1, n_chunks], mybir.dt.int32, isOutput=False)
    dcol_p = nc.declare_dram_parameter("dcol", [P, t_total], bf16, isOutput=False)
    iota_p = nc.declare_dram_parameter("iota", [P, P], bf16, isOutput=False)
    out_p = nc.declare_dram_parameter("out", [NPC, D_FEAT], f32, isOutput=True)

    nblk = tq.shape[0]
    tiles = tq.sum(axis=1)
    t_max = int(tiles.max())

    # group chunks by block for the build loop; gi values within one
    # block are contiguous (chunks were built block-major), which the
    # 4-wide register preload below relies on
    chunks_by_block = [[] for _ in range(nblk)]
    for gi, (b, q, done, ch) in enumerate(chunks):
        if chunks_by_block[b]:
            assert gi == chunks_by_block[b][-1][0] + 1
        chunks_by_block[b].append((gi, q, done, ch))

    with tile.TileContext(nc) as tc:
        with (
            nc.gpsimd.register("vreg0") as vreg0,
            nc.gpsimd.register("vreg1") as vreg1,
            nc.gpsimd.register("vreg2") as vreg2,
            nc.gpsimd.register("vreg3") as vreg3,
            tc.tile_pool(name="const", bufs=1) as const_tp,
            tc.tile_pool(name="meta", bufs=1) as meta_tp,
            tc.tile_pool(name="gather", bufs=6) as gather_tp,
            tc.tile_pool(name="sel", bufs=3) as sel_tp,
            tc.tile_pool(name="xin", bufs=3) as xin_tp,
            tc.tile_pool(name="osb", bufs=3) as osb_tp,
            tc.tile_pool(name="psum", bufs=4, space="PSUM") as psum_tp,
        ):
            iota_sb = const_tp.tile([P, P], bf16)
            nc.sync.dma_start(out=iota_sb[:], in_=iota_p[:])
            idx_sb = meta_tp.tile([P, t_total * 8], mybir.dt.int16)
            nc.sync.dma_start(out=idx_sb[:], in_=srci_p[:])
            col_sb = meta_tp.tile([P, t_total], bf16)
            nc.sync.dma_start(out=col_sb[:], in_=dcol_p[:])
            vc_sb = meta_tp.tile([1, n_chunks], mybir.dt.int32)
            nc.sync.dma_start(out=vc_sb[:], in_=vc_p[:])

            # memset the gather pool slots once: slots the -1-padded
            # gathers skip must hold finite data for the 0*x matmuls
            for _ in range(6):
                gz = gather_tp.tile([P, t_max * ROWPAD], bf16, tag="g")
                nc.vector.memset(gz[:], 0.0)

            gather_i = 0
            t0 = 0
            for b in range(nblk):
                tb = int(tiles[b])
                rows = min(P, NPC - b * P)

                g = gather_tp.tile([P, t_max * ROWPAD], bf16, tag="g")
                qoff = np.zeros(NQUAD, dtype=int)
                qoff[1:] = np.cumsum(tq[b])[:-1]
                vregs = [vreg0, vreg1, vreg2, vreg3]
                for ci, (gi, q, done, ch) in enumerate(chunks_by_block[b]):
                    co = int(qoff[q]) + done
                    o16 = (t0 + co) * 8
                    if ci % 4 == 0:
                        nb = min(4, len(chunks_by_block[b]) - ci)
                        nc.gpsimd.load(
                            vregs[:nb], vc_sb[0:1, gi : gi + nb]
                        )
                    vreg = vregs[ci % 4]
                    nc.gpsimd.dma_gather(
                        out_ap=g[
                            :, co * ROWPAD : (co + ch) * ROWPAD
                        ].rearrange("p (c d) -> p c d", d=ROWPAD),
                        in_ap=xq_p[q][:, :],
                        idxs_ap=idx_sb[:, o16 : o16 + ch * 8],
                        num_idxs=ch * P,
                        num_idxs_reg=vreg,
                        elem_size=ROWPAD,
                        queue_num=gather_i % 4,
                        single_packet=False,
                    )
                    gather_i += 1

                sel = sel_tp.tile([P, t_max * P], bf16, tag="s")
                nc.vector.tensor_tensor(
                    out=sel[:, : tb * P].rearrange("p (t n) -> p t n", n=P),
                    in0=col_sb[:, t0 : t0 + tb].unsqueeze(2).to_broadcast(
                        [P, tb, P]
                    ),
                    in1=iota_sb[:].unsqueeze(1).to_broadcast([P, tb, P]),
                    op=mybir.AluOpType.is_equal,
                )

                ps = psum_tp.tile([P, D_FEAT], f32, space="PSUM", tag="ps")
                for t in range(tb):
                    nc.tensor.matmul(
                        out=ps[:],
                        lhsT=sel[:, t * P : (t + 1) * P],
                        rhs=g[:, t * ROWPAD : t * ROWPAD + D_FEAT],
                        start=(t == 0),
                        stop=(t == tb - 1),
                    )

                xt = xin_tp.tile([P, D_FEAT], f32, tag="x")
                nc.sync.dma_start(out=xt[:rows], in_=xsl_p[b * P : b * P + rows, :])
                ot = osb_tp.tile([P, D_FEAT], f32, tag="o")
                nc.vector.tensor_add(out=ot[:rows], in0=xt[:rows], in1=ps[:rows])
                nc.sync.dma_start(out=out_p[b * P : b * P + rows, :], in_=ot[:rows])

                t0 += tb
    nc.compile()
    return nc


def kernel(x, edge_index):
    global LAST_EXEC_TIME_NS
    _patch_tile_drain()

    x = np.ascontiguousarray(np.asarray(x, dtype=np.float32))
    idx_arr, col_arr, tq, chunks, vc_arr, t_total = _preprocess(x, edge_index)

    xq = np.zeros((N_NODES, ROWPAD), dtype=ml_dtypes.bfloat16)
    xq[:, :D_FEAT] = (x * np.float32(WEIGHT)).astype(ml_dtypes.bfloat16)
    iota = np.broadcast_to(
        np.arange(P, dtype=np.float32).astype(ml_dtypes.bfloat16), (P, P)
    ).copy()

    nc = _build_graph(tq, chunks, t_total)

    in_maps = []
    for c in range(N_CORES):
        m = {
            "xsl": np.ascontiguousarray(x[c * NPC : (c + 1) * NPC]),
            "srci": np.ascontiguousarray(idx_arr[c]),
            "dcol": np.ascontiguousarray(col_arr[c]),
            "vc": np.ascontiguousarray(vc_arr[c : c + 1]),
            "iota": iota,
        }
        for q in range(NQUAD):
            m[f"xq{q}"] = np.ascontiguousarray(
                xq[q * NQROWS : (q + 1) * NQROWS]
            )
        in_maps.append(m)

    trace = bool(os.environ.get("BASS_KERNEL_TRACE"))
    if trace:
        _install_ntff_hook()
    res = run_bass_kernel_spmd(
        nc, in_maps, core_ids=list(range(N_CORES)), trace=trace
    )
    LAST_EXEC_TIME_NS = res.exec_time_ns

    out = np.concatenate([res.results[c]["out"] for c in range(N_CORES)], axis=0)
    return out.astype(np.float32)


# revision 19
# speedup vs baseline: 1.1247x; 1.1247x over previous
"""Trainium2 Bass kernel for GNN message passing (APPR-style aggregation).

Computes: out = x + 0.15 * segment_sum(x[src], dst, num_segments=N)
for x [100000, 64] f32 and edge_index [2, 1600000] int64.

Strategy (8 NeuronCores, no collectives needed):
  - Host shards EDGES by destination-owner core (core c owns nodes
    [c*12500, (c+1)*12500)); within a core, edges are bucketed by
    128-node destination block and by source quadrant (x split into 4
    row-quadrants so dma_gather's int16 indices can address it). This
    makes the aggregation node-sharded from the start, so each core
    independently produces its slice of the output.
  - On device, per 128-node block: dma_gather of 0.15*x[src] rows (bf16,
    rows padded to 256 B) into SBUF, one gather per source quadrant,
    round-robined over all 4 SWDGE queues; build one-hot selection
    matrices S[e, j] = (dstcol_e == j) with a DVE is_equal against an
    iota tile, and accumulate S^T @ gathered into an f32 PSUM tile over
    all edge tiles of the block. Epilogue adds the f32 x slice and DMAs
    the block out.
  - All 8 cores run the same static graph: per-(block, quadrant) tile
    counts are maxed across cores; each core pads its edge list with
    valid index-0 entries up to the shared valid count and with -1
    entries (skipped by the gather) up to the tile boundary. Pad slots
    carry dstcol=255 which matches no one-hot column, so they contribute
    zero; slots the gather skips hold stale-but-finite SBUF data (the
    gather pool is memset once at startup).

  Perf notes (measured): the bottleneck is GPSIMD Q7 descriptor emission
  for the gathers (~3.5 ns/row with 4 SWDGE queues; 1 queue is ~3x
  slower). dma_gather is capped at 1024 indices per instruction.
  TensorE/VectorE/DMA all hide under the gather wall.
"""

import os
import sys
import types

import numpy as np

for _p in ("/opt/trn_rl_repo", "/root/.axon_site/_ro/trn_rl_repo"):
    if os.path.isdir(_p) and _p not in sys.path:
        sys.path.append(_p)

import ml_dtypes
import concourse.bass as bass
import concourse.mybir as mybir
import concourse.tile as tile
from concourse import bacc
from concourse.bass_utils import run_bass_kernel_spmd
from concourse.vector_clock import ScopedClock

WEIGHT = 0.15
N_NODES = 100000
D_FEAT = 64
N_CORES = 8
P = 128
NQUAD = 4
ROWPAD = 128  # gathered bf16 row padded to 128 elems = 256 B
NPC = N_NODES // N_CORES  # nodes per core
NBLK = (NPC + P - 1) // P  # 128-node dst blocks per core
NQROWS = N_NODES // NQUAD  # rows per source quadrant (must fit int16)

LAST_EXEC_TIME_NS = None

# Emit only up to the max-over-cores valid count per gather and pad the
# rest with -1 (skipped; slots keep stale-but-finite SBUF data). The
# CoreSim interpreter poisons unwritten regions of the gather's out view,
# so simulation-based tests set this to False.
TRAILING_SKIP = True

MAX_WAITS = 2  # this walrus build rejects instructions with more sync commands


def _patch_tile_drain():
    """This walrus build rejects >MAX_WAITS sync commands (waits+updates)
    on one instruction. Two patches: (a) the tail drain re-emits its waits
    as individual wait_ge instructions; (b) any scheduled instruction with
    too many waits gets the excess hoisted onto same-engine InstNoOps
    placed immediately before it."""
    if getattr(tile.TileContext, "_drain_patched", False):
        return

    def _drain_and_barrier(self, tick_clock, wait_clock):
        drain_inst = self.nc.sync.drain()
        wait_clock.add_sem_waits(
            drain_inst.ins, ScopedClock({None: tick_clock.global_clock})
        )
        si = drain_inst.ins.sync_info
        waits = list(si.on_wait) if si is not None else []
        if len(waits) > MAX_WAITS:
            drain_inst.ins.sync_info = mybir.SyncInfo(on_wait=[], on_update=[])
            handles = {h.name: h for h in wait_clock.sems.allocated().values()}
            for w in waits:
                self.nc.sync.wait_ge(handles[w.ant_name], w.wait_value)
            self.nc.sync.drain()
        self.nc.all_engine_barrier()
        popped = self.nc._tile_sem_poison_stack.pop()
        assert popped is self._sem_poison
        self.nc.clear_and_free_semaphores(list(self.sems.allocated().values()))
        self.nc.all_engine_barrier()

    orig_lower = tile.TileContext._lower_ordered_insts

    def _lower_ordered_insts(self, ordered):
        for bb_name, insts in ordered.items():
            new_list = []
            for inst in insts:
                si = getattr(inst, "sync_info", None)
                n_w = len(si.on_wait) if si is not None and si.on_wait else 0
                n_u = len(si.on_update) if si is not None and si.on_update else 0
                budget = max(0, MAX_WAITS - n_u)
                if (
                    n_w > budget
                    and type(inst).__name__.startswith("Inst")
                    and inst.engine is not None
                ):
                    waits = list(si.on_wait)
                    keep = waits[len(waits) - budget :] if budget else []
                    excess = waits[: len(waits) - budget]
                    for w in excess:
                        nop = mybir.InstNoOp(
                            name=self.nc.get_next_instruction_name(),
                            sync_info=mybir.SyncInfo(on_wait=[w], on_update=[]),
                            engine=inst.engine,
                            bass_nofuse=True,
                        )
                        new_list.append(nop)
                    inst.sync_info = mybir.SyncInfo(
                        on_wait=keep, on_update=list(si.on_update)
                    )
                new_list.append(inst)
            insts[:] = new_list
        return orig_lower(self, ordered)

    tile.TileContext._drain_and_barrier = _drain_and_barrier
    tile.TileContext._lower_ordered_insts = _lower_ordered_insts
    tile.TileContext._drain_patched = True


def _install_ntff_hook():
    """Register the NTFF profiling hook that this container's boot skips
    (antenv.axon_hooks missing). Only needed when tracing is requested."""
    if "antenv.axon_hooks" in sys.modules:
        return
    try:
        from trn_agent_boot.trn_boot import _ntff_profile_via_ctypes

        hook = _ntff_profile_via_ctypes("/opt/axon/libaxon_pjrt.so")
        if hook is None:
            return
        mod = types.ModuleType("antenv.axon_hooks")
        mod._hook = hook
        mod.get_axon_ntff_profile_hook = lambda: mod._hook
        mod.set_axon_ntff_profile_hook = lambda h: setattr(mod, "_hook", h)
        sys.modules["antenv.axon_hooks"] = mod
        import antenv

        antenv.axon_hooks = mod
    except Exception as e:  # profiling is optional
        print(f"ntff hook install failed: {e}", file=sys.stderr)


def _preprocess(x, edge_index):
    """Bucket edges per (core, dst-block, src-quadrant); build device
    input arrays in the slot order the device graph consumes."""
    src = np.asarray(edge_index[0]).astype(np.int64)
    dst = np.asarray(edge_index[1]).astype(np.int64)
    E = src.shape[0]

    core = dst // NPC
    local = dst - core * NPC
    blk = local >> 7
    col = local & 127
    quad = src // NQROWS
    loc = (src - quad * NQROWS).astype(np.int64)

    gkey = (core * NBLK + blk) * NQUAD + quad
    order = np.argsort(gkey, kind="stable")
    gkey_s = gkey[order]
    loc_s = loc[order]
    col_s = col[order]

    counts = np.bincount(gkey, minlength=N_CORES * NBLK * NQUAD).reshape(
        N_CORES, NBLK, NQUAD
    )
    maxc = counts.max(axis=0)  # [NBLK, NQUAD] max edges over cores
    tq = (maxc + P - 1) // P  # tiles per (block, quadrant)
    for b in range(NBLK):
        if tq[b].sum() == 0:
            tq[b, 0] = 1
    tiles = tq.sum(axis=1)
    t_total = int(tiles.sum())

    toff = np.zeros((NBLK, NQUAD), dtype=np.int64)
    toff.ravel()[1:] = np.cumsum(tq.ravel())[:-1]

    group_starts = np.zeros(N_CORES * NBLK * NQUAD + 1, dtype=np.int64)
    np.cumsum(counts.ravel(), out=group_starts[1:])
    j = np.arange(E) - group_starts[gkey_s]  # slot within group
    bq_s = gkey_s % (NBLK * NQUAD)
    core_s = gkey_s // (NBLK * NQUAD)
    base_tile = toff.ravel()[bq_s]

    # col array: [core, 128, t_total]; slot i -> lane i%128, tile base+i//128
    col_arr = np.full((N_CORES, P, t_total), 255.0, dtype=ml_dtypes.bfloat16)
    col_arr[core_s, j & 127, base_tile + (j >> 7)] = col_s.astype(
        ml_dtypes.bfloat16
    )

    # Gather chunks: dma_gather is capped at 8 tiles (1024 idxs); each
    # (b, q) group is split into <=8-tile chunks. Build the per-core,
    # per-chunk valid counts (fed to num_idxs_reg via a register) and the
    # idx arrays: each core's real edges, then -1s (skipped) to the tile
    # boundary. A chunk with zero valid entries for some core gets one
    # valid pad entry (index 0) so the gather ucode never sees count 0.
    t16_total = t_total * 8
    fill = -1 if TRAILING_SKIP else 0
    idx16 = np.full((N_CORES, 16, t16_total), fill, dtype=np.int16)
    idx16[core_s, j & 15, base_tile * 8 + (j >> 4)] = loc_s

    chunks = []  # (b, q, tile_offset_in_block, n_tiles, chunk_idx_global)
    vc_list = []  # per chunk: [N_CORES] valid counts
    for b in range(NBLK):
        for q in range(NQUAD):
            tn = int(tq[b, q])
            done = 0
            while done < tn:
                ch = min(8, tn - done)
                lo, hi = done * P, done * P + ch * P
                v = np.clip(counts[:, b, q] - lo, 0, ch * P)
                if TRAILING_SKIP:
                    need_pad = v == 0
                    if need_pad.any():
                        o16 = (int(toff[b, q]) + done) * 8
                        idx16[need_pad, 0, o16] = 0
                    v = np.maximum(v, 1)
                else:
                    v[:] = ch * P
                chunks.append((b, q, done, ch))
                vc_list.append(v)
                done += ch
    vc_arr = np.stack(vc_list, axis=1).astype(np.int32)  # [N_CORES, G]
    idx_arr = np.tile(idx16, (1, 8, 1))

    return idx_arr, col_arr, tq.astype(int), chunks, vc_arr, t_total


def _build_graph(tq, chunks, t_total):
    nc = bacc.Bacc(num_swdge_queues=4, dynamic_dma_scratch_size=65536)
    f32 = mybir.dt.float32
    bf16 = mybir.dt.bfloat16
    xq_p = [
        nc.declare_dram_parameter(f"xq{q}", [NQROWS, ROWPAD], bf16, isOutput=False)
        for q in range(NQUAD)
    ]
    xsl_p = nc.declare_dram_parameter("xsl", [NPC, D_FEAT], f32, isOutput=False)
    srci_p = nc.declare_dram_parameter(
        "srci", [P, t_total * 8], mybir.dt.int16, isOutput=False
    )
    n_chunks = len(chunks)
    vc_p = nc.declare_dram_parameter("vc", [1, n_chunks], mybir.dt.int32, isOutput=False)
    dcol_p = nc.declare_dram_parameter("dcol", [P, t_total], bf16, isOutput=False)
    iota_p = nc.declare_dram_parameter("iota", [P, P], bf16, isOutput=False)
    out_p = nc.declare_dram_parameter("out", [NPC, D_FEAT], f32, isOutput=True)

    nblk = tq.shape[0]
    tiles = tq.sum(axis=1)
    t_max = int(tiles.max())

    # group chunks by block for the build loop; gi values within one
    # block are contiguous (chunks were built block-major), which the
    # 4-wide register preload below relies on
    chunks_by_block = [[] for _ in range(nblk)]
    for gi, (b, q, done, ch) in enumerate(chunks):
        if chunks_by_block[b]:
            assert gi == chunks_by_block[b][-1][0] + 1
        chunks_by_block[b].append((gi, q, done, ch))

    with tile.TileContext(nc) as tc:
        with (
            nc.gpsimd.register("vreg0") as vreg0,
            nc.gpsimd.register("vreg1") as vreg1,
            nc.gpsimd.register("vreg2") as vreg2,
            nc.gpsimd.register("vreg3") as vreg3,
            tc.tile_pool(name="const", bufs=1) as const_tp,
            tc.tile_pool(name="meta", bufs=1) as meta_tp,
            tc.tile_pool(name="gather", bufs=6) as gather_tp,
            tc.tile_pool(name="sel", bufs=3) as sel_tp,
            tc.tile_pool(name="xin", bufs=3) as xin_tp,
            tc.tile_pool(name="osb", bufs=3) as osb_tp,
            tc.tile_pool(name="psum", bufs=4, space="PSUM") as psum_tp,
        ):
            iota_sb = const_tp.tile([P, P], bf16)
            nc.sync.dma_start(out=iota_sb[:], in_=iota_p[:])
            vc_sb = meta_tp.tile([1, n_chunks], mybir.dt.int32)
            nc.sync.dma_start(out=vc_sb[:], in_=vc_p[:])

            # metadata is loaded in 8 chunks of blocks so the first
            # gathers start without waiting for the whole 4 MB load
            NMETA = min(8, nblk)
            blk_ranges = np.array_split(np.arange(nblk), NMETA)
            blk_rng_of = np.zeros(nblk, dtype=int)
            rng_t0 = np.zeros(NMETA, dtype=int)
            rng_tn = np.zeros(NMETA, dtype=int)
            tile_starts = np.zeros(nblk + 1, dtype=int)
            np.cumsum(tiles, out=tile_starts[1:])
            idx_tiles, col_tiles = [], []
            for r, blks in enumerate(blk_ranges):
                blk_rng_of[blks] = r
                rng_t0[r] = tile_starts[blks[0]]
                rng_tn[r] = int(tiles[blks].sum())
                it = meta_tp.tile([P, int(rng_tn[r]) * 8], mybir.dt.int16, tag=f"idx{r}")
                nc.sync.dma_start(
                    out=it[:],
                    in_=srci_p[:, int(rng_t0[r]) * 8 : (int(rng_t0[r]) + int(rng_tn[r])) * 8],
                )
                idx_tiles.append(it)
                ct = meta_tp.tile([P, int(rng_tn[r])], bf16, tag=f"col{r}")
                nc.sync.dma_start(
                    out=ct[:],
                    in_=dcol_p[:, int(rng_t0[r]) : int(rng_t0[r]) + int(rng_tn[r])],
                )
                col_tiles.append(ct)

            # memset the gather pool slots once: slots the -1-padded
            # gathers skip must hold finite data for the 0*x matmuls
            for _ in range(6):
                gz = gather_tp.tile([P, t_max * ROWPAD], bf16, tag="g")
                nc.vector.memset(gz[:], 0.0)

            gather_i = 0
            t0 = 0
            for b in range(nblk):
                tb = int(tiles[b])
                rows = min(P, NPC - b * P)

                g = gather_tp.tile([P, t_max * ROWPAD], bf16, tag="g")
                qoff = np.zeros(NQUAD, dtype=int)
                qoff[1:] = np.cumsum(tq[b])[:-1]
                vregs = [vreg0, vreg1, vreg2, vreg3]
                r = int(blk_rng_of[b])
                idx_sb = idx_tiles[r]
                rt0 = int(rng_t0[r])
                for ci, (gi, q, done, ch) in enumerate(chunks_by_block[b]):
                    co = int(qoff[q]) + done
                    o16 = (t0 - rt0 + co) * 8
                    if ci % 4 == 0:
                        nb = min(4, len(chunks_by_block[b]) - ci)
                        nc.gpsimd.load(
                            vregs[:nb], vc_sb[0:1, gi : gi + nb]
                        )
                    vreg = vregs[ci % 4]
                    nc.gpsimd.dma_gather(
                        out_ap=g[
                            :, co * ROWPAD : (co + ch) * ROWPAD
                        ].rearrange("p (c d) -> p c d", d=ROWPAD),
                        in_ap=xq_p[q][:, :],
                        idxs_ap=idx_sb[:, o16 : o16 + ch * 8],
                        num_idxs=ch * P,
                        num_idxs_reg=vreg,
                        elem_size=ROWPAD,
                        queue_num=gather_i % 4,
                        single_packet=False,
                    )
                    gather_i += 1

                sel = sel_tp.tile([P, t_max * P], bf16, tag="s")
                nc.vector.tensor_tensor(
                    out=sel[:, : tb * P].rearrange("p (t n) -> p t n", n=P),
                    in0=col_tiles[r][:, t0 - rt0 : t0 - rt0 + tb]
                    .unsqueeze(2)
                    .to_broadcast([P, tb, P]),
                    in1=iota_sb[:].unsqueeze(1).to_broadcast([P, tb, P]),
                    op=mybir.AluOpType.is_equal,
                )

                ps = psum_tp.tile([P, D_FEAT], f32, space="PSUM", tag="ps")
                for t in range(tb):
                    nc.tensor.matmul(
                        out=ps[:],
                        lhsT=sel[:, t * P : (t + 1) * P],
                        rhs=g[:, t * ROWPAD : t * ROWPAD + D_FEAT],
                        start=(t == 0),
                        stop=(t == tb - 1),
                    )

                xt = xin_tp.tile([P, D_FEAT], f32, tag="x")
                nc.sync.dma_start(out=xt[:rows], in_=xsl_p[b * P : b * P + rows, :])
                ot = osb_tp.tile([P, D_FEAT], f32, tag="o")
                nc.vector.tensor_add(out=ot[:rows], in0=xt[:rows], in1=ps[:rows])
                nc.sync.dma_start(out=out_p[b * P : b * P + rows, :], in_=ot[:rows])

                t0 += tb
    nc.compile()
    return nc


def kernel(x, edge_index):
    global LAST_EXEC_TIME_NS
    _patch_tile_drain()

    x = np.ascontiguousarray(np.asarray(x, dtype=np.float32))
    idx_arr, col_arr, tq, chunks, vc_arr, t_total = _preprocess(x, edge_index)

    xq = np.zeros((N_NODES, ROWPAD), dtype=ml_dtypes.bfloat16)
    xq[:, :D_FEAT] = (x * np.float32(WEIGHT)).astype(ml_dtypes.bfloat16)
    iota = np.broadcast_to(
        np.arange(P, dtype=np.float32).astype(ml_dtypes.bfloat16), (P, P)
    ).copy()

    nc = _build_graph(tq, chunks, t_total)

    in_maps = []
    for c in range(N_CORES):
        m = {
            "xsl": np.ascontiguousarray(x[c * NPC : (c + 1) * NPC]),
            "srci": np.ascontiguousarray(idx_arr[c]),
            "dcol": np.ascontiguousarray(col_arr[c]),
            "vc": np.ascontiguousarray(vc_arr[c : c + 1]),
            "iota": iota,
        }
        for q in range(NQUAD):
            m[f"xq{q}"] = np.ascontiguousarray(
                xq[q * NQROWS : (q + 1) * NQROWS]
            )
        in_maps.append(m)

    trace = bool(os.environ.get("BASS_KERNEL_TRACE"))
    if trace:
        _install_ntff_hook()
    res = run_bass_kernel_spmd(
        nc, in_maps, core_ids=list(range(N_CORES)), trace=trace
    )
    LAST_EXEC_TIME_NS = res.exec_time_ns

    out = np.concatenate([res.results[c]["out"] for c in range(N_CORES)], axis=0)
    return out.astype(np.float32)


# revision 20
# speedup vs baseline: 1.1613x; 1.0325x over previous
"""Trainium2 Bass kernel for GNN message passing (APPR-style aggregation).

Computes: out = x + 0.15 * segment_sum(x[src], dst, num_segments=N)
for x [100000, 64] f32 and edge_index [2, 1600000] int64.

Strategy (8 NeuronCores, no collectives needed):
  - Host shards EDGES by destination-owner core (core c owns nodes
    [c*12500, (c+1)*12500)); within a core, edges are bucketed by
    128-node destination block and by source quadrant (x split into 4
    row-quadrants so dma_gather's int16 indices can address it). This
    makes the aggregation node-sharded from the start, so each core
    independently produces its slice of the output.
  - On device, per 128-node block: dma_gather of 0.15*x[src] rows (bf16,
    rows padded to 256 B) into SBUF, one gather per source quadrant,
    round-robined over all 4 SWDGE queues; build one-hot selection
    matrices S[e, j] = (dstcol_e == j) with a DVE is_equal against an
    iota tile, and accumulate S^T @ gathered into an f32 PSUM tile over
    all edge tiles of the block. Epilogue adds the f32 x slice and DMAs
    the block out.
  - All 8 cores run the same static graph: per-(block, quadrant) tile
    counts are maxed across cores; each core pads its edge list with
    valid index-0 entries up to the shared valid count and with -1
    entries (skipped by the gather) up to the tile boundary. Pad slots
    carry dstcol=255 which matches no one-hot column, so they contribute
    zero; slots the gather skips hold stale-but-finite SBUF data (the
    gather pool is memset once at startup).

  Perf notes (measured): the bottleneck is GPSIMD Q7 descriptor emission
  for the gathers (~3.5 ns/row with 4 SWDGE queues; 1 queue is ~3x
  slower). dma_gather is capped at 1024 indices per instruction.
  TensorE/VectorE/DMA all hide under the gather wall.
"""

import os
import sys
import types

import numpy as np

for _p in ("/opt/trn_rl_repo", "/root/.axon_site/_ro/trn_rl_repo"):
    if os.path.isdir(_p) and _p not in sys.path:
        sys.path.append(_p)

import ml_dtypes
import concourse.bass as bass
import concourse.mybir as mybir
import concourse.tile as tile
from concourse import bacc
from concourse.bass_utils import run_bass_kernel_spmd
from concourse.vector_clock import ScopedClock

WEIGHT = 0.15
N_NODES = 100000
D_FEAT = 64
N_CORES = 8
P = 128
NQUAD = 4
ROWPAD = 128  # gathered bf16 row padded to 128 elems = 256 B
NPC = N_NODES // N_CORES  # nodes per core
NBLK = (NPC + P - 1) // P  # 128-node dst blocks per core
NQROWS = N_NODES // NQUAD  # rows per source quadrant (must fit int16)

LAST_EXEC_TIME_NS = None

GATHER_BUFS = 6  # gather pool slots; first GATHER_BUFS blocks emit all slots

# Emit only up to the max-over-cores valid count per gather and pad the
# rest with -1 (skipped; slots keep stale-but-finite SBUF data). The
# CoreSim interpreter poisons unwritten regions of the gather's out view,
# so simulation-based tests set this to False.
TRAILING_SKIP = True

MAX_WAITS = 2  # this walrus build rejects instructions with more sync commands


def _patch_tile_drain():
    """This walrus build rejects >MAX_WAITS sync commands (waits+updates)
    on one instruction. Two patches: (a) the tail drain re-emits its waits
    as individual wait_ge instructions; (b) any scheduled instruction with
    too many waits gets the excess hoisted onto same-engine InstNoOps
    placed immediately before it."""
    if getattr(tile.TileContext, "_drain_patched", False):
        return

    def _drain_and_barrier(self, tick_clock, wait_clock):
        drain_inst = self.nc.sync.drain()
        wait_clock.add_sem_waits(
            drain_inst.ins, ScopedClock({None: tick_clock.global_clock})
        )
        si = drain_inst.ins.sync_info
        waits = list(si.on_wait) if si is not None else []
        if len(waits) > MAX_WAITS:
            drain_inst.ins.sync_info = mybir.SyncInfo(on_wait=[], on_update=[])
            handles = {h.name: h for h in wait_clock.sems.allocated().values()}
            for w in waits:
                self.nc.sync.wait_ge(handles[w.ant_name], w.wait_value)
            self.nc.sync.drain()
        self.nc.all_engine_barrier()
        popped = self.nc._tile_sem_poison_stack.pop()
        assert popped is self._sem_poison
        self.nc.clear_and_free_semaphores(list(self.sems.allocated().values()))
        self.nc.all_engine_barrier()

    orig_lower = tile.TileContext._lower_ordered_insts

    def _lower_ordered_insts(self, ordered):
        for bb_name, insts in ordered.items():
            new_list = []
            for inst in insts:
                si = getattr(inst, "sync_info", None)
                n_w = len(si.on_wait) if si is not None and si.on_wait else 0
                n_u = len(si.on_update) if si is not None and si.on_update else 0
                budget = max(0, MAX_WAITS - n_u)
                if (
                    n_w > budget
                    and type(inst).__name__.startswith("Inst")
                    and inst.engine is not None
                ):
                    waits = list(si.on_wait)
                    keep = waits[len(waits) - budget :] if budget else []
                    excess = waits[: len(waits) - budget]
                    for w in excess:
                        nop = mybir.InstNoOp(
                            name=self.nc.get_next_instruction_name(),
                            sync_info=mybir.SyncInfo(on_wait=[w], on_update=[]),
                            engine=inst.engine,
                            bass_nofuse=True,
                        )
                        new_list.append(nop)
                    inst.sync_info = mybir.SyncInfo(
                        on_wait=keep, on_update=list(si.on_update)
                    )
                new_list.append(inst)
            insts[:] = new_list
        return orig_lower(self, ordered)

    tile.TileContext._drain_and_barrier = _drain_and_barrier
    tile.TileContext._lower_ordered_insts = _lower_ordered_insts
    tile.TileContext._drain_patched = True


def _install_ntff_hook():
    """Register the NTFF profiling hook that this container's boot skips
    (antenv.axon_hooks missing). Only needed when tracing is requested."""
    if "antenv.axon_hooks" in sys.modules:
        return
    try:
        from trn_agent_boot.trn_boot import _ntff_profile_via_ctypes

        hook = _ntff_profile_via_ctypes("/opt/axon/libaxon_pjrt.so")
        if hook is None:
            return
        mod = types.ModuleType("antenv.axon_hooks")
        mod._hook = hook
        mod.get_axon_ntff_profile_hook = lambda: mod._hook
        mod.set_axon_ntff_profile_hook = lambda h: setattr(mod, "_hook", h)
        sys.modules["antenv.axon_hooks"] = mod
        import antenv

        antenv.axon_hooks = mod
    except Exception as e:  # profiling is optional
        print(f"ntff hook install failed: {e}", file=sys.stderr)


def _preprocess(x, edge_index):
    """Bucket edges per (core, dst-block, src-quadrant); build device
    input arrays in the slot order the device graph consumes."""
    src = np.asarray(edge_index[0]).astype(np.int64)
    dst = np.asarray(edge_index[1]).astype(np.int64)
    E = src.shape[0]

    core = dst // NPC
    local = dst - core * NPC
    blk = local >> 7
    col = local & 127
    quad = src // NQROWS
    loc = (src - quad * NQROWS).astype(np.int64)

    gkey = (core * NBLK + blk) * NQUAD + quad
    order = np.argsort(gkey, kind="stable")
    gkey_s = gkey[order]
    loc_s = loc[order]
    col_s = col[order]

    counts = np.bincount(gkey, minlength=N_CORES * NBLK * NQUAD).reshape(
        N_CORES, NBLK, NQUAD
    )
    maxc = counts.max(axis=0)  # [NBLK, NQUAD] max edges over cores
    tq = (maxc + P - 1) // P  # tiles per (block, quadrant)
    for b in range(NBLK):
        if tq[b].sum() == 0:
            tq[b, 0] = 1
    tiles = tq.sum(axis=1)
    t_total = int(tiles.sum())

    toff = np.zeros((NBLK, NQUAD), dtype=np.int64)
    toff.ravel()[1:] = np.cumsum(tq.ravel())[:-1]

    group_starts = np.zeros(N_CORES * NBLK * NQUAD + 1, dtype=np.int64)
    np.cumsum(counts.ravel(), out=group_starts[1:])
    j = np.arange(E) - group_starts[gkey_s]  # slot within group
    bq_s = gkey_s % (NBLK * NQUAD)
    core_s = gkey_s // (NBLK * NQUAD)
    base_tile = toff.ravel()[bq_s]

    # col array: [core, 128, t_total]; slot i -> lane i%128, tile base+i//128
    col_arr = np.full((N_CORES, P, t_total), 255.0, dtype=ml_dtypes.bfloat16)
    col_arr[core_s, j & 127, base_tile + (j >> 7)] = col_s.astype(
        ml_dtypes.bfloat16
    )

    # Gather chunks: dma_gather is capped at 8 tiles (1024 idxs); each
    # (b, q) group is split into <=8-tile chunks. Build the per-core,
    # per-chunk valid counts (fed to num_idxs_reg via a register) and the
    # idx arrays: each core's real edges, then -1s (skipped) to the tile
    # boundary. A chunk with zero valid entries for some core gets one
    # valid pad entry (index 0) so the gather ucode never sees count 0.
    t16_total = t_total * 8
    fill = -1 if TRAILING_SKIP else 0
    idx16 = np.full((N_CORES, 16, t16_total), fill, dtype=np.int16)
    idx16[core_s, j & 15, base_tile * 8 + (j >> 4)] = loc_s

    # The first GATHER_BUFS blocks emit every slot (index-0 padding, no
    # -1 skips): they are each pool slot's first writer, so afterwards
    # skipped slots always hold finite data from a previous gather.
    chunks = []  # (b, q, tile_offset_in_block, n_tiles)
    vc_list = []  # per chunk: [N_CORES] valid counts
    for b in range(NBLK):
        full = (not TRAILING_SKIP) or b < GATHER_BUFS
        for q in range(NQUAD):
            tn = int(tq[b, q])
            done = 0
            while done < tn:
                ch = min(8, tn - done)
                lo = done * P
                v = np.clip(counts[:, b, q] - lo, 0, ch * P)
                if full:
                    o16 = (int(toff[b, q]) + done) * 8
                    idx16[:, :, o16 : o16 + ch * 8][
                        idx16[:, :, o16 : o16 + ch * 8] == -1
                    ] = 0
                    v[:] = ch * P
                else:
                    need_pad = v == 0
                    if need_pad.any():
                        o16 = (int(toff[b, q]) + done) * 8
                        idx16[need_pad, 0, o16] = 0
                    v = np.maximum(v, 1)
                chunks.append((b, q, done, ch))
                vc_list.append(v)
                done += ch
    vc_arr = np.stack(vc_list, axis=1).astype(np.int32)  # [N_CORES, G]
    idx_arr = np.tile(idx16, (1, 8, 1))

    return idx_arr, col_arr, tq.astype(int), chunks, vc_arr, t_total


def _build_graph(tq, chunks, t_total):
    nc = bacc.Bacc(num_swdge_queues=4, dynamic_dma_scratch_size=65536)
    f32 = mybir.dt.float32
    bf16 = mybir.dt.bfloat16
    xq_p = [
        nc.declare_dram_parameter(f"xq{q}", [NQROWS, ROWPAD], bf16, isOutput=False)
        for q in range(NQUAD)
    ]
    xsl_p = nc.declare_dram_parameter("xsl", [NPC, D_FEAT], f32, isOutput=False)
    srci_p = nc.declare_dram_parameter(
        "srci", [P, t_total * 8], mybir.dt.int16, isOutput=False
    )
    n_chunks = len(chunks)
    vc_p = nc.declare_dram_parameter("vc", [1, n_chunks], mybir.dt.int32, isOutput=False)
    dcol_p = nc.declare_dram_parameter("dcol", [P, t_total], bf16, isOutput=False)
    iota_p = nc.declare_dram_parameter("iota", [P, P], bf16, isOutput=False)
    out_p = nc.declare_dram_parameter("out", [NPC, D_FEAT], f32, isOutput=True)

    nblk = tq.shape[0]
    tiles = tq.sum(axis=1)
    t_max = int(tiles.max())

    # group chunks by block for the build loop; gi values within one
    # block are contiguous (chunks were built block-major), which the
    # 4-wide register preload below relies on
    chunks_by_block = [[] for _ in range(nblk)]
    for gi, (b, q, done, ch) in enumerate(chunks):
        if chunks_by_block[b]:
            assert gi == chunks_by_block[b][-1][0] + 1
        chunks_by_block[b].append((gi, q, done, ch))

    with tile.TileContext(nc) as tc:
        with (
            nc.gpsimd.register("vreg0") as vreg0,
            nc.gpsimd.register("vreg1") as vreg1,
            nc.gpsimd.register("vreg2") as vreg2,
            nc.gpsimd.register("vreg3") as vreg3,
            tc.tile_pool(name="const", bufs=1) as const_tp,
            tc.tile_pool(name="meta", bufs=1) as meta_tp,
            tc.tile_pool(name="gather", bufs=GATHER_BUFS) as gather_tp,
            tc.tile_pool(name="sel", bufs=3) as sel_tp,
            tc.tile_pool(name="xin", bufs=3) as xin_tp,
            tc.tile_pool(name="osb", bufs=3) as osb_tp,
            tc.tile_pool(name="psum", bufs=4, space="PSUM") as psum_tp,
        ):
            iota_sb = const_tp.tile([P, P], bf16)
            nc.sync.dma_start(out=iota_sb[:], in_=iota_p[:])
            vc_sb = meta_tp.tile([1, n_chunks], mybir.dt.int32)
            nc.sync.dma_start(out=vc_sb[:], in_=vc_p[:])

            # metadata is loaded in 8 chunks of blocks so the first
            # gathers start without waiting for the whole 4 MB load
            NMETA = min(8, nblk)
            blk_ranges = np.array_split(np.arange(nblk), NMETA)
            blk_rng_of = np.zeros(nblk, dtype=int)
            rng_t0 = np.zeros(NMETA, dtype=int)
            rng_tn = np.zeros(NMETA, dtype=int)
            tile_starts = np.zeros(nblk + 1, dtype=int)
            np.cumsum(tiles, out=tile_starts[1:])
            idx_tiles, col_tiles = [], []
            for r, blks in enumerate(blk_ranges):
                blk_rng_of[blks] = r
                rng_t0[r] = tile_starts[blks[0]]
                rng_tn[r] = int(tiles[blks].sum())
                it = meta_tp.tile([P, int(rng_tn[r]) * 8], mybir.dt.int16, tag=f"idx{r}")
                nc.sync.dma_start(
                    out=it[:],
                    in_=srci_p[:, int(rng_t0[r]) * 8 : (int(rng_t0[r]) + int(rng_tn[r])) * 8],
                )
                idx_tiles.append(it)
                ct = meta_tp.tile([P, int(rng_tn[r])], bf16, tag=f"col{r}")
                nc.sync.dma_start(
                    out=ct[:],
                    in_=dcol_p[:, int(rng_t0[r]) : int(rng_t0[r]) + int(rng_tn[r])],
                )
                col_tiles.append(ct)

            gather_i = 0
            t0 = 0
            for b in range(nblk):
                tb = int(tiles[b])
                rows = min(P, NPC - b * P)

                g = gather_tp.tile([P, t_max * ROWPAD], bf16, tag="g")
                qoff = np.zeros(NQUAD, dtype=int)
                qoff[1:] = np.cumsum(tq[b])[:-1]
                vregs = [vreg0, vreg1, vreg2, vreg3]
                r = int(blk_rng_of[b])
                idx_sb = idx_tiles[r]
                rt0 = int(rng_t0[r])
                for ci, (gi, q, done, ch) in enumerate(chunks_by_block[b]):
                    co = int(qoff[q]) + done
                    o16 = (t0 - rt0 + co) * 8
                    if ci % 4 == 0:
                        nb = min(4, len(chunks_by_block[b]) - ci)
                        nc.gpsimd.load(
                            vregs[:nb], vc_sb[0:1, gi : gi + nb]
                        )
                    vreg = vregs[ci % 4]
                    nc.gpsimd.dma_gather(
                        out_ap=g[
                            :, co * ROWPAD : (co + ch) * ROWPAD
                        ].rearrange("p (c d) -> p c d", d=ROWPAD),
                        in_ap=xq_p[q][:, :],
                        idxs_ap=idx_sb[:, o16 : o16 + ch * 8],
                        num_idxs=ch * P,
                        num_idxs_reg=vreg,
                        elem_size=ROWPAD,
                        queue_num=gather_i % 4,
                        single_packet=False,
                    )
                    gather_i += 1

                sel = sel_tp.tile([P, t_max * P], bf16, tag="s")
                nc.vector.tensor_tensor(
                    out=sel[:, : tb * P].rearrange("p (t n) -> p t n", n=P),
                    in0=col_tiles[r][:, t0 - rt0 : t0 - rt0 + tb]
                    .unsqueeze(2)
                    .to_broadcast([P, tb, P]),
                    in1=iota_sb[:].unsqueeze(1).to_broadcast([P, tb, P]),
                    op=mybir.AluOpType.is_equal,
                )

                ps = psum_tp.tile([P, D_FEAT], f32, space="PSUM", tag="ps")
                for t in range(tb):
                    nc.tensor.matmul(
                        out=ps[:],
                        lhsT=sel[:, t * P : (t + 1) * P],
                        rhs=g[:, t * ROWPAD : t * ROWPAD + D_FEAT],
                        start=(t == 0),
                        stop=(t == tb - 1),
                    )

                xt = xin_tp.tile([P, D_FEAT], f32, tag="x")
                nc.sync.dma_start(out=xt[:rows], in_=xsl_p[b * P : b * P + rows, :])
                ot = osb_tp.tile([P, D_FEAT], f32, tag="o")
                nc.vector.tensor_add(out=ot[:rows], in0=xt[:rows], in1=ps[:rows])
                nc.sync.dma_start(out=out_p[b * P : b * P + rows, :], in_=ot[:rows])

                t0 += tb
    nc.compile()
    return nc


def kernel(x, edge_index):
    global LAST_EXEC_TIME_NS
    _patch_tile_drain()

    x = np.ascontiguousarray(np.asarray(x, dtype=np.float32))
    idx_arr, col_arr, tq, chunks, vc_arr, t_total = _preprocess(x, edge_index)

    xq = np.zeros((N_NODES, ROWPAD), dtype=ml_dtypes.bfloat16)
    xq[:, :D_FEAT] = (x * np.float32(WEIGHT)).astype(ml_dtypes.bfloat16)
    iota = np.broadcast_to(
        np.arange(P, dtype=np.float32).astype(ml_dtypes.bfloat16), (P, P)
    ).copy()

    nc = _build_graph(tq, chunks, t_total)

    in_maps = []
    for c in range(N_CORES):
        m = {
            "xsl": np.ascontiguousarray(x[c * NPC : (c + 1) * NPC]),
            "srci": np.ascontiguousarray(idx_arr[c]),
            "dcol": np.ascontiguousarray(col_arr[c]),
            "vc": np.ascontiguousarray(vc_arr[c : c + 1]),
            "iota": iota,
        }
        for q in range(NQUAD):
            m[f"xq{q}"] = np.ascontiguousarray(
                xq[q * NQROWS : (q + 1) * NQROWS]
            )
        in_maps.append(m)

    trace = bool(os.environ.get("BASS_KERNEL_TRACE"))
    if trace:
        _install_ntff_hook()
    res = run_bass_kernel_spmd(
        nc, in_maps, core_ids=list(range(N_CORES)), trace=trace
    )
    LAST_EXEC_TIME_NS = res.exec_time_ns

    out = np.concatenate([res.results[c]["out"] for c in range(N_CORES)], axis=0)
    return out.astype(np.float32)


# revision 21
# speedup vs baseline: 1.1805x; 1.0165x over previous
"""Trainium2 Bass kernel for GNN message passing (APPR-style aggregation).

Computes: out = x + 0.15 * segment_sum(x[src], dst, num_segments=N)
for x [100000, 64] f32 and edge_index [2, 1600000] int64.

Strategy (8 NeuronCores, no collectives needed):
  - Host shards EDGES by destination-owner core (core c owns nodes
    [c*12500, (c+1)*12500)); within a core, edges are bucketed by
    128-node destination block and by source quadrant (x split into 4
    row-quadrants so dma_gather's int16 indices can address it). This
    makes the aggregation node-sharded from the start, so each core
    independently produces its slice of the output.
  - On device, per 128-node block: dma_gather of 0.15*x[src] rows (bf16,
    rows padded to 256 B) into SBUF, one gather per source quadrant,
    round-robined over all 4 SWDGE queues; build one-hot selection
    matrices S[e, j] = (dstcol_e == j) with a DVE is_equal against an
    iota tile, and accumulate S^T @ gathered into an f32 PSUM tile over
    all edge tiles of the block. Epilogue adds the f32 x slice and DMAs
    the block out.
  - All 8 cores run the same static graph: per-(block, quadrant) tile
    counts are maxed across cores; each core pads its edge list with
    valid index-0 entries up to the shared valid count and with -1
    entries (skipped by the gather) up to the tile boundary. Pad slots
    carry dstcol=255 which matches no one-hot column, so they contribute
    zero; slots the gather skips hold stale-but-finite SBUF data (the
    gather pool is memset once at startup).

  Perf notes (measured): the bottleneck is GPSIMD Q7 descriptor emission
  for the gathers (~3.5 ns/row with 4 SWDGE queues; 1 queue is ~3x
  slower). dma_gather is capped at 1024 indices per instruction.
  TensorE/VectorE/DMA all hide under the gather wall.
"""

import os
import sys
import types

import numpy as np

for _p in ("/opt/trn_rl_repo", "/root/.axon_site/_ro/trn_rl_repo"):
    if os.path.isdir(_p) and _p not in sys.path:
        sys.path.append(_p)

import ml_dtypes
import concourse.bass as bass
import concourse.mybir as mybir
import concourse.tile as tile
from concourse import bacc
from concourse.bass_utils import run_bass_kernel_spmd
from concourse.vector_clock import ScopedClock

WEIGHT = 0.15
N_NODES = 100000
D_FEAT = 64
N_CORES = 8
P = 128
NQUAD = 4
ROWPAD = 128  # gathered bf16 row padded to 128 elems = 256 B
NPC = N_NODES // N_CORES  # nodes per core
NBLK = (NPC + P - 1) // P  # 128-node dst blocks per core
NQROWS = N_NODES // NQUAD  # rows per source quadrant (must fit int16)

LAST_EXEC_TIME_NS = None

GATHER_BUFS = 8  # gather pool slots; first GATHER_BUFS blocks emit all slots

# Emit only up to the max-over-cores valid count per gather and pad the
# rest with -1 (skipped; slots keep stale-but-finite SBUF data). The
# CoreSim interpreter poisons unwritten regions of the gather's out view,
# so simulation-based tests set this to False.
TRAILING_SKIP = True

MAX_WAITS = 2  # this walrus build rejects instructions with more sync commands


def _patch_tile_drain():
    """This walrus build rejects >MAX_WAITS sync commands (waits+updates)
    on one instruction. Two patches: (a) the tail drain re-emits its waits
    as individual wait_ge instructions; (b) any scheduled instruction with
    too many waits gets the excess hoisted onto same-engine InstNoOps
    placed immediately before it."""
    if getattr(tile.TileContext, "_drain_patched", False):
        return

    def _drain_and_barrier(self, tick_clock, wait_clock):
        drain_inst = self.nc.sync.drain()
        wait_clock.add_sem_waits(
            drain_inst.ins, ScopedClock({None: tick_clock.global_clock})
        )
        si = drain_inst.ins.sync_info
        waits = list(si.on_wait) if si is not None else []
        if len(waits) > MAX_WAITS:
            drain_inst.ins.sync_info = mybir.SyncInfo(on_wait=[], on_update=[])
            handles = {h.name: h for h in wait_clock.sems.allocated().values()}
            for w in waits:
                self.nc.sync.wait_ge(handles[w.ant_name], w.wait_value)
            self.nc.sync.drain()
        self.nc.all_engine_barrier()
        popped = self.nc._tile_sem_poison_stack.pop()
        assert popped is self._sem_poison
        self.nc.clear_and_free_semaphores(list(self.sems.allocated().values()))
        self.nc.all_engine_barrier()

    orig_lower = tile.TileContext._lower_ordered_insts

    def _lower_ordered_insts(self, ordered):
        for bb_name, insts in ordered.items():
            new_list = []
            for inst in insts:
                si = getattr(inst, "sync_info", None)
                n_w = len(si.on_wait) if si is not None and si.on_wait else 0
                n_u = len(si.on_update) if si is not None and si.on_update else 0
                budget = max(0, MAX_WAITS - n_u)
                if (
                    n_w > budget
                    and type(inst).__name__.startswith("Inst")
                    and inst.engine is not None
                ):
                    waits = list(si.on_wait)
                    keep = waits[len(waits) - budget :] if budget else []
                    excess = waits[: len(waits) - budget]
                    for w in excess:
                        nop = mybir.InstNoOp(
                            name=self.nc.get_next_instruction_name(),
                            sync_info=mybir.SyncInfo(on_wait=[w], on_update=[]),
                            engine=inst.engine,
                            bass_nofuse=True,
                        )
                        new_list.append(nop)
                    inst.sync_info = mybir.SyncInfo(
                        on_wait=keep, on_update=list(si.on_update)
                    )
                new_list.append(inst)
            insts[:] = new_list
        return orig_lower(self, ordered)

    tile.TileContext._drain_and_barrier = _drain_and_barrier
    tile.TileContext._lower_ordered_insts = _lower_ordered_insts
    tile.TileContext._drain_patched = True


def _install_ntff_hook():
    """Register the NTFF profiling hook that this container's boot skips
    (antenv.axon_hooks missing). Only needed when tracing is requested."""
    if "antenv.axon_hooks" in sys.modules:
        return
    try:
        from trn_agent_boot.trn_boot import _ntff_profile_via_ctypes

        hook = _ntff_profile_via_ctypes("/opt/axon/libaxon_pjrt.so")
        if hook is None:
            return
        mod = types.ModuleType("antenv.axon_hooks")
        mod._hook = hook
        mod.get_axon_ntff_profile_hook = lambda: mod._hook
        mod.set_axon_ntff_profile_hook = lambda h: setattr(mod, "_hook", h)
        sys.modules["antenv.axon_hooks"] = mod
        import antenv

        antenv.axon_hooks = mod
    except Exception as e:  # profiling is optional
        print(f"ntff hook install failed: {e}", file=sys.stderr)


def _preprocess(x, edge_index):
    """Bucket edges per (core, dst-block, src-quadrant); build device
    input arrays in the slot order the device graph consumes."""
    src = np.asarray(edge_index[0]).astype(np.int64)
    dst = np.asarray(edge_index[1]).astype(np.int64)
    E = src.shape[0]

    core = dst // NPC
    local = dst - core * NPC
    blk = local >> 7
    col = local & 127
    quad = src // NQROWS
    loc = (src - quad * NQROWS).astype(np.int64)

    gkey = (core * NBLK + blk) * NQUAD + quad
    order = np.argsort(gkey, kind="stable")
    gkey_s = gkey[order]
    loc_s = loc[order]
    col_s = col[order]

    counts = np.bincount(gkey, minlength=N_CORES * NBLK * NQUAD).reshape(
        N_CORES, NBLK, NQUAD
    )
    maxc = counts.max(axis=0)  # [NBLK, NQUAD] max edges over cores
    tq = (maxc + P - 1) // P  # tiles per (block, quadrant)
    for b in range(NBLK):
        if tq[b].sum() == 0:
            tq[b, 0] = 1
    tiles = tq.sum(axis=1)
    t_total = int(tiles.sum())

    toff = np.zeros((NBLK, NQUAD), dtype=np.int64)
    toff.ravel()[1:] = np.cumsum(tq.ravel())[:-1]

    group_starts = np.zeros(N_CORES * NBLK * NQUAD + 1, dtype=np.int64)
    np.cumsum(counts.ravel(), out=group_starts[1:])
    j = np.arange(E) - group_starts[gkey_s]  # slot within group
    bq_s = gkey_s % (NBLK * NQUAD)
    core_s = gkey_s // (NBLK * NQUAD)
    base_tile = toff.ravel()[bq_s]

    # col array: [core, 128, t_total]; slot i -> lane i%128, tile base+i//128
    col_arr = np.full((N_CORES, P, t_total), 255.0, dtype=ml_dtypes.bfloat16)
    col_arr[core_s, j & 127, base_tile + (j >> 7)] = col_s.astype(
        ml_dtypes.bfloat16
    )

    # Gather chunks: dma_gather is capped at 8 tiles (1024 idxs); each
    # (b, q) group is split into <=8-tile chunks. Build the per-core,
    # per-chunk valid counts (fed to num_idxs_reg via a register) and the
    # idx arrays: each core's real edges, then -1s (skipped) to the tile
    # boundary. A chunk with zero valid entries for some core gets one
    # valid pad entry (index 0) so the gather ucode never sees count 0.
    t16_total = t_total * 8
    fill = -1 if TRAILING_SKIP else 0
    idx16 = np.full((N_CORES, 16, t16_total), fill, dtype=np.int16)
    idx16[core_s, j & 15, base_tile * 8 + (j >> 4)] = loc_s

    # The first GATHER_BUFS blocks emit every slot (index-0 padding, no
    # -1 skips): they are each pool slot's first writer, so afterwards
    # skipped slots always hold finite data from a previous gather.
    chunks = []  # (b, q, tile_offset_in_block, n_tiles)
    vc_list = []  # per chunk: [N_CORES] valid counts
    for b in range(NBLK):
        full = (not TRAILING_SKIP) or b < GATHER_BUFS
        for q in range(NQUAD):
            tn = int(tq[b, q])
            done = 0
            while done < tn:
                ch = min(8, tn - done)
                lo = done * P
                v = np.clip(counts[:, b, q] - lo, 0, ch * P)
                if full:
                    o16 = (int(toff[b, q]) + done) * 8
                    idx16[:, :, o16 : o16 + ch * 8][
                        idx16[:, :, o16 : o16 + ch * 8] == -1
                    ] = 0
                    v[:] = ch * P
                else:
                    need_pad = v == 0
                    if need_pad.any():
                        o16 = (int(toff[b, q]) + done) * 8
                        idx16[need_pad, 0, o16] = 0
                    v = np.maximum(v, 1)
                chunks.append((b, q, done, ch))
                vc_list.append(v)
                done += ch
    vc_arr = np.stack(vc_list, axis=1).astype(np.int32)  # [N_CORES, G]
    idx_arr = np.tile(idx16, (1, 8, 1))

    return idx_arr, col_arr, tq.astype(int), chunks, vc_arr, t_total


def _build_graph(tq, chunks, t_total):
    nc = bacc.Bacc(num_swdge_queues=4, dynamic_dma_scratch_size=65536)
    f32 = mybir.dt.float32
    bf16 = mybir.dt.bfloat16
    xq_p = [
        nc.declare_dram_parameter(f"xq{q}", [NQROWS, ROWPAD], bf16, isOutput=False)
        for q in range(NQUAD)
    ]
    xsl_p = nc.declare_dram_parameter("xsl", [NPC, D_FEAT], f32, isOutput=False)
    srci_p = nc.declare_dram_parameter(
        "srci", [P, t_total * 8], mybir.dt.int16, isOutput=False
    )
    n_chunks = len(chunks)
    vc_p = nc.declare_dram_parameter("vc", [1, n_chunks], mybir.dt.int32, isOutput=False)
    dcol_p = nc.declare_dram_parameter("dcol", [P, t_total], bf16, isOutput=False)
    iota_p = nc.declare_dram_parameter("iota", [P, P], bf16, isOutput=False)
    out_p = nc.declare_dram_parameter("out", [NPC, D_FEAT], f32, isOutput=True)

    nblk = tq.shape[0]
    tiles = tq.sum(axis=1)
    t_max = int(tiles.max())

    # group chunks by block for the build loop; gi values within one
    # block are contiguous (chunks were built block-major), which the
    # 4-wide register preload below relies on
    chunks_by_block = [[] for _ in range(nblk)]
    for gi, (b, q, done, ch) in enumerate(chunks):
        if chunks_by_block[b]:
            assert gi == chunks_by_block[b][-1][0] + 1
        chunks_by_block[b].append((gi, q, done, ch))

    with tile.TileContext(nc) as tc:
        with (
            nc.gpsimd.register("vreg0") as vreg0,
            nc.gpsimd.register("vreg1") as vreg1,
            nc.gpsimd.register("vreg2") as vreg2,
            nc.gpsimd.register("vreg3") as vreg3,
            tc.tile_pool(name="const", bufs=1) as const_tp,
            tc.tile_pool(name="meta", bufs=1) as meta_tp,
            tc.tile_pool(name="gather", bufs=GATHER_BUFS) as gather_tp,
            tc.tile_pool(name="sel", bufs=4) as sel_tp,
            tc.tile_pool(name="xin", bufs=3) as xin_tp,
            tc.tile_pool(name="osb", bufs=3) as osb_tp,
            tc.tile_pool(name="psum", bufs=4, space="PSUM") as psum_tp,
        ):
            iota_sb = const_tp.tile([P, P], bf16)
            nc.sync.dma_start(out=iota_sb[:], in_=iota_p[:])
            vc_sb = meta_tp.tile([1, n_chunks], mybir.dt.int32)
            nc.sync.dma_start(out=vc_sb[:], in_=vc_p[:])

            # metadata is loaded in 8 chunks of blocks so the first
            # gathers start without waiting for the whole 4 MB load
            NMETA = min(8, nblk)
            blk_ranges = np.array_split(np.arange(nblk), NMETA)
            blk_rng_of = np.zeros(nblk, dtype=int)
            rng_t0 = np.zeros(NMETA, dtype=int)
            rng_tn = np.zeros(NMETA, dtype=int)
            tile_starts = np.zeros(nblk + 1, dtype=int)
            np.cumsum(tiles, out=tile_starts[1:])
            idx_tiles, col_tiles = [], []
            for r, blks in enumerate(blk_ranges):
                blk_rng_of[blks] = r
                rng_t0[r] = tile_starts[blks[0]]
                rng_tn[r] = int(tiles[blks].sum())
                it = meta_tp.tile([P, int(rng_tn[r]) * 8], mybir.dt.int16, tag=f"idx{r}")
                nc.sync.dma_start(
                    out=it[:],
                    in_=srci_p[:, int(rng_t0[r]) * 8 : (int(rng_t0[r]) + int(rng_tn[r])) * 8],
                )
                idx_tiles.append(it)
                ct = meta_tp.tile([P, int(rng_tn[r])], bf16, tag=f"col{r}")
                nc.sync.dma_start(
                    out=ct[:],
                    in_=dcol_p[:, int(rng_t0[r]) : int(rng_t0[r]) + int(rng_tn[r])],
                )
                col_tiles.append(ct)

            gather_i = 0
            t0 = 0
            for b in range(nblk):
                tb = int(tiles[b])
                rows = min(P, NPC - b * P)

                g = gather_tp.tile([P, t_max * ROWPAD], bf16, tag="g")
                qoff = np.zeros(NQUAD, dtype=int)
                qoff[1:] = np.cumsum(tq[b])[:-1]
                vregs = [vreg0, vreg1, vreg2, vreg3]
                r = int(blk_rng_of[b])
                idx_sb = idx_tiles[r]
                rt0 = int(rng_t0[r])
                for ci, (gi, q, done, ch) in enumerate(chunks_by_block[b]):
                    co = int(qoff[q]) + done
                    o16 = (t0 - rt0 + co) * 8
                    if ci % 4 == 0:
                        nb = min(4, len(chunks_by_block[b]) - ci)
                        nc.gpsimd.load(
                            vregs[:nb], vc_sb[0:1, gi : gi + nb]
                        )
                    vreg = vregs[ci % 4]
                    nc.gpsimd.dma_gather(
                        out_ap=g[
                            :, co * ROWPAD : (co + ch) * ROWPAD
                        ].rearrange("p (c d) -> p c d", d=ROWPAD),
                        in_ap=xq_p[q][:, :],
                        idxs_ap=idx_sb[:, o16 : o16 + ch * 8],
                        num_idxs=ch * P,
                        num_idxs_reg=vreg,
                        elem_size=ROWPAD,
                        queue_num=gather_i % 4,
                        single_packet=False,
                    )
                    gather_i += 1

                sel = sel_tp.tile([P, t_max * P], bf16, tag="s")
                nc.vector.tensor_tensor(
                    out=sel[:, : tb * P].rearrange("p (t n) -> p t n", n=P),
                    in0=col_tiles[r][:, t0 - rt0 : t0 - rt0 + tb]
                    .unsqueeze(2)
                    .to_broadcast([P, tb, P]),
                    in1=iota_sb[:].unsqueeze(1).to_broadcast([P, tb, P]),
                    op=mybir.AluOpType.is_equal,
                )

                ps = psum_tp.tile([P, D_FEAT], f32, space="PSUM", tag="ps")
                for t in range(tb):
                    nc.tensor.matmul(
                        out=ps[:],
                        lhsT=sel[:, t * P : (t + 1) * P],
                        rhs=g[:, t * ROWPAD : t * ROWPAD + D_FEAT],
                        start=(t == 0),
                        stop=(t == tb - 1),
                    )

                xt = xin_tp.tile([P, D_FEAT], f32, tag="x")
                nc.sync.dma_start(out=xt[:rows], in_=xsl_p[b * P : b * P + rows, :])
                ot = osb_tp.tile([P, D_FEAT], f32, tag="o")
                nc.vector.tensor_add(out=ot[:rows], in0=xt[:rows], in1=ps[:rows])
                nc.sync.dma_start(out=out_p[b * P : b * P + rows, :], in_=ot[:rows])

                t0 += tb
    nc.compile()
    return nc


def kernel(x, edge_index):
    global LAST_EXEC_TIME_NS
    _patch_tile_drain()

    x = np.ascontiguousarray(np.asarray(x, dtype=np.float32))
    idx_arr, col_arr, tq, chunks, vc_arr, t_total = _preprocess(x, edge_index)

    xq = np.zeros((N_NODES, ROWPAD), dtype=ml_dtypes.bfloat16)
    xq[:, :D_FEAT] = (x * np.float32(WEIGHT)).astype(ml_dtypes.bfloat16)
    iota = np.broadcast_to(
        np.arange(P, dtype=np.float32).astype(ml_dtypes.bfloat16), (P, P)
    ).copy()

    nc = _build_graph(tq, chunks, t_total)

    in_maps = []
    for c in range(N_CORES):
        m = {
            "xsl": np.ascontiguousarray(x[c * NPC : (c + 1) * NPC]),
            "srci": np.ascontiguousarray(idx_arr[c]),
            "dcol": np.ascontiguousarray(col_arr[c]),
            "vc": np.ascontiguousarray(vc_arr[c : c + 1]),
            "iota": iota,
        }
        for q in range(NQUAD):
            m[f"xq{q}"] = np.ascontiguousarray(
                xq[q * NQROWS : (q + 1) * NQROWS]
            )
        in_maps.append(m)

    trace = bool(os.environ.get("BASS_KERNEL_TRACE"))
    if trace:
        _install_ntff_hook()
    res = run_bass_kernel_spmd(
        nc, in_maps, core_ids=list(range(N_CORES)), trace=trace
    )
    LAST_EXEC_TIME_NS = res.exec_time_ns

    out = np.concatenate([res.results[c]["out"] for c in range(N_CORES)], axis=0)
    return out.astype(np.float32)
